# revision 39
# baseline (speedup 1.0000x reference)
"""Joint Maximum Mean Discrepancy loss on 8 Trainium2 NeuronCores.

Math: for streams (s0,t0) and (s1,t1), the reference builds per-stream
Gaussian kernels K_r = exp(-gamma_r * dist_r) over feats_r = [src; tgt]
(N=8192 rows), takes their elementwise product, and returns
mean(s2s + t2t - 2*s2t) over the B x B blocks.

Device decomposition (fp8 DoubleRow matmuls + exp on ScalarE + block
sums on VectorE):
  exponent E_ij = 2*W_i.W_j - c_i - c_j with W = [sqrt(g0)*X0,
  sqrt(g1)*X1] (N x 320), c_i = |W_i|^2. Split c = cbar + delta and
  scale everything by 16 so the fp8 operands sit in e4m3's normal
  range:
    P_ij = (8W0_i).(4W0_j) + (8W1_i).(4W1_j) - 16*delta_i - 16*delta_j
    K_ij = exp(P_ij/16 - 2*cbar)
  The 256 stream-0 rows go through ONE fp8e4 DoubleRow matmul per
  m-tile (K=256 virtual in a single 512-col pass, ~2x bf16); the
  stream-1 rows + two delta rows ride a 66-row bf16 matmul. -2*cbar is
  the activation's per-partition bias AP and the 1/16 its scale, so the
  scalar engine applies them for free inside exp. Quantizing W in fp8
  perturbs source and target features identically, so the MMD
  difference cancels the quantization error (measured end-to-end rel
  err ~1e-5, on par with all-bf16).

  Symmetry halves the work via a block-cyclic cover: core k owns
  row-chunks {k, k+8} (chunk = 512 rows) and computes 17 blocks —
  column offsets d=0..8 from row chunk k, d=0..7 from chunk k+8 —
  every unordered off-diagonal chunk pair once (weight 2, sign from
  the src/tgt halves), diagonals once. Off-diagonal block: 8 matmuls
  (4 m-tiles x fp8-DR + bf16) into a 4-bank PSUM tile, one Exp
  activation [128, 2048] psum -> SBUF fp16 (releasing psum at ACT end
  keeps the PE dense), and one two-port scalar_tensor_tensor on the
  vector engine summing the two fp16 halves into an acc column (2048
  elements in ~1024 DVE cycles, off the critical path). The two
  DIAGONAL chunk-blocks use a triangular cover: only n>=m tiles,
  packed contiguously into psum [0:1280) (the m=2 run split at the
  bank boundary), one 1280-col Exp, a packed sum S and a strided
  diag-tile-only sum D; the host reconstructs the block as 2*S - D.

  Steady state is the 2-slot psum pipeline: period = (MM 1.83us +
  ACT 1.97us + 2 sem hops)/2 ~ 2.0us/block. The scalar engine's exp
  throughput (1 elem/cycle/lane @1.2GHz) is the hard floor.

  HAM choreography (empirical, this silicon): only the rapid-fire
  burst of small matmuls issued immediately at kernel start reliably
  flips the PE clock to 2.4 GHz; a burst after a cold data-wait, or
  plain N=512 matmuls, leave it at 1.2 GHz, and any sizeable PE idle
  after the warm transition re-throttles and STICKS cold for tens of
  microseconds. Hence: warmup burst first (covers the input-DMA
  latency), first processed block is a regular one whose chunk leads
  the gpsimd queue, diagonal blocks run 3rd and last.

Per-core device program (SPMD — identical instructions, data differs):
  - lhs8 [2, 128, 2, 512] fp8e4 : DoubleRow stationary, [g][p,t,r] =
    8*W0[gbase+r, t*128+p]
  - lhsb [2, 66, 512] bf16     : [8*W1^T ; ones ; -16*delta]
  - rhs8 [8, 128, 2, 2, 512] fp8e4 : chunk-PAIR tiles (one DMA each),
    [j][p,u,t,n] = 4*W0[(2j+u)*512+n, t*128+p], chunk order rotated by
    k so the program's chunk index is core-local
  - rhsb [8, 66, 2, 512] bf16  : [4*W1^T ; -16*delta ; ones]
  - bias [128, 1] f32 = -2*cbar
  - out "acc" [128, 19] f32 (17 block sums + 2 diag-tile sums)
"""

import os

import numpy as np
import ml_dtypes

import concourse.bacc as bacc
import concourse.bass as bass
import concourse.mybir as mybir
import concourse.tile as tile
from concourse.bass_utils import run_bass_kernel_spmd

B = 4096
D0, D1 = 256, 64
N = 2 * B
CH = 512          # rows per chunk
NCHUNK = 16
NCORE = 8
MT = 128          # m-tile rows / partition count
NMT = CH // MT    # m-tiles per row-chunk (4)
NBLK = 17         # blocks per core (9 from chunk k, 8 from chunk k+8)
NCOL = NBLK + 3   # 17 block sums + 2 diag-tile sums + block-0 2nd half
KB = D1 + 2       # bf16 contraction rows: 64 stream-1 + delta_j + delta_i
LAM = 4.0         # fp8 range scale on each operand; exp rescales by 1/16

F8 = ml_dtypes.float8_e4m3
BF = ml_dtypes.bfloat16

_N_WARMUP = int(os.environ.get("JMMD_WARMUP", "36"))

LAST_EXEC_NS = None
LAST_RESULTS = None

_CACHE: dict = {}


def _build():
    if "nc" in _CACHE:
        return _CACHE["nc"]
    nc = bacc.Bacc(
        "TRN2", target_bir_lowering=False, debug=False, enable_asserts=False
    )
    f32 = mybir.dt.float32
    bf16 = mybir.dt.bfloat16
    f8 = mybir.dt.float8e4
    DR = mybir.MatmulPerfMode.DoubleRow

    lhs8_d = nc.dram_tensor("lhs8", [2, MT, 2, CH], f8, kind="ExternalInput").ap()
    lhsb_d = nc.dram_tensor("lhsb", [2, KB, CH], bf16, kind="ExternalInput").ap()
    rhs8_d = nc.dram_tensor("rhs8", [8, MT, 2, 2, CH], f8, kind="ExternalInput").ap()
    rhsb_d = nc.dram_tensor("rhsb", [8, KB, 2, CH], bf16, kind="ExternalInput").ap()
    bias_d = nc.dram_tensor("bias", [MT, 1], f32, kind="ExternalInput").ap()
    acc_d = nc.dram_tensor("acc", [MT, NCOL], f32, kind="ExternalOutput").ap()

    with tile.TileContext(nc) as tc:
        with (
            tc.tile_pool(name="const", bufs=1) as const,
            tc.tile_pool(name="psum", bufs=2, space=bass.MemorySpace.PSUM) as psum,
        ):
            # warmup scratch memset on the idle vector engine so both DMA
            # queues start issuing transfers immediately.
            scratch = const.tile([MT, 256], bf16, tag="warm_src")
            nc.vector.memset(scratch[:], 0.0)

            l8, lb, r8, rb = {}, {}, {}, {}

            def load_lhs(g, eng):
                t8 = const.tile([MT, 2, CH], f8, tag=f"l8_{g}")
                eng.dma_start(t8[:], lhs8_d[g])
                l8[g] = t8
                tb = const.tile([KB, CH], bf16, tag=f"lb_{g}")
                eng.dma_start(tb[:], lhsb_d[g])
                lb[g] = tb

            def load_rhs(j, eng):
                t8 = const.tile([MT, 2, 2, CH], f8, tag=f"r8_{j}")
                eng.dma_start(t8[:], rhs8_d[j])
                r8[j] = t8
                tb = const.tile([KB, 2, CH], bf16, tag=f"rb_{j}")
                eng.dma_start(tb[:], rhsb_d[j])
                rb[j] = tb

            # Block 0's operands race down both DMA engines in parallel —
            # chunk pair 0 is split into per-chunk pieces so ch0 lands as
            # early as possible (any idle gap between the HAM warmup and
            # the first real matmuls re-throttles the PE clock, and it has
            # been observed to stay stuck at 1.2 GHz for the whole kernel).
            # The first processed block is (g=0, d=1) = local chunk 1 —
            # its rhs piece leads the gpsimd queue while sync carries
            # bias + lhs + chunk 0 (the diagonal block runs 3rd, once the
            # DMA stream is comfortably ahead).
            r8_0 = const.tile([MT, 2, 2, CH], f8, tag="r8_0")
            rb_0 = const.tile([KB, 2, CH], bf16, tag="rb_0")
            nc.gpsimd.dma_start(r8_0[:, 1], rhs8_d[0, :, 1])
            nc.gpsimd.dma_start(rb_0[:, 1], rhsb_d[0, :, 1])
            r8[0], rb[0] = r8_0, rb_0
            bias_t = const.tile([MT, 1], f32, tag="bias")
            nc.sync.dma_start(bias_t[:], bias_d[:, :])
            load_lhs(0, nc.sync)
            nc.sync.dma_start(r8_0[:, 0], rhs8_d[0, :, 0])
            nc.sync.dma_start(rb_0[:, 0], rhsb_d[0, :, 0])
            load_rhs(1, nc.gpsimd)
            load_rhs(2, nc.sync)
            load_rhs(3, nc.gpsimd)
            load_lhs(1, nc.sync)
            load_rhs(4, nc.sync)
            load_rhs(5, nc.gpsimd)
            load_rhs(6, nc.sync)
            load_rhs(7, nc.gpsimd)

            acc_t = const.tile([MT, NCOL], f32, tag="acc")
            # exp lands in SBUF (fp16) so the psum tile frees at ACT end —
            # keeping the PE dense — and the vector engine sums it off the
            # critical path with a two-port tensor_tensor_reduce (2048
            # elements in ~1024 cycles).
            exp_t = const.tile([MT, 2, NMT * CH], mybir.dt.float16, tag="exp")
            red_t = const.tile([MT, NMT * CH // 2], mybir.dt.float16, tag="red")

            # HAM choreography, learned the hard way on this silicon: the
            # rapid-fire burst of small matmuls issued IMMEDIATELY at
            # kernel start reliably flips the PE clock to 2.4 GHz (4/4
            # runs) — whereas a burst issued after a cold data-wait, or
            # dense N=512 real matmuls alone, leave the clock stuck at
            # 1.2 GHz for tens of microseconds (0/4 runs). The burst also
            # covers the input-DMA latency, and the real block stream
            # behind it must then accumulate no more than ~2us of PE idle
            # per ~3.4us window or the clock re-throttles and sticks.
            warm_ps = psum.tile([MT, NMT * CH], f32, tag="ps")
            for _ in range(_N_WARMUP):
                nc.tensor.matmul(
                    warm_ps[:, :MT],
                    scratch[:, :MT],
                    scratch[:, MT:],
                    start=True,
                    stop=True,
                )

            EXP = mybir.ActivationFunctionType.Exp
            MULT, ADD = mybir.AluOpType.mult, mybir.AluOpType.add

            def stt_sum(in0, in1, acc_col, out=None):
                if out is None:
                    out = red_t[:, :in0.shape[-1]]
                nc.vector.scalar_tensor_tensor(
                    out,
                    in0,
                    1.0,
                    in1,
                    op0=MULT,
                    op1=ADD,
                    accum_out=acc_t[:, acc_col:acc_col + 1],
                )

            # Diagonal chunk-blocks (d=0 for both g) use a triangular
            # cover: only n-tiles n>=m are computed, packed CONTIGUOUSLY
            # at psum [0:1280) (m-runs 512/384/256/128; the m=2 run is
            # emitted as two 128-col matmul pairs so no single matmul
            # output crosses a psum bank boundary). One 1280-col Exp then
            # beats the regular 2048-col one by ~600ns on the binding
            # scalar engine. Host reconstructs the full block as 2*S - D
            # where S is the packed sum (acc col) and D the diag-tile-only
            # sum (acc col 17+g, gathered via two strided 2x128 views).
            DRUNS = [  # (m, col_lo, col_hi, psum_offset)
                (0, 0, CH, 0),
                (1, MT, CH, CH),
                (2, 2 * MT, 3 * MT, CH + 3 * MT),
                (2, 3 * MT, CH, 2 * CH),
                (3, 3 * MT, CH, 2 * CH + MT),
            ]
            DTOT = 2 * CH + MT * 2                   # 1280 packed columns

            blocks = [(0, d) for d in (1, 2, 0, 3, 4, 5, 6, 7, 8)]
            blocks += [(1, d) for d in (1, 2, 3, 4, 5, 6, 7, 0)]
            for bi, (g, d) in enumerate(blocks):
                    ch = d if g == 0 else 8 + d
                    col = d if g == 0 else 9 + d
                    j, u = divmod(ch, 2)
                    diag = d == 0
                    ps = psum.tile([MT, NMT * CH], f32, tag="ps")
                    runs = DRUNS if diag else [
                        (m, 0, CH, m * CH) for m in range(NMT)
                    ]
                    # Interleaved DR/bf16 pairs per run: alternating the
                    # 256-col DR weight loads with cheap bf16 ones keeps
                    # LDWEIGHTS hidden behind the matmul streaming (a
                    # DR-first ordering stacks 213ns weight loads that
                    # cannot hide behind 241ns DR matmuls and costs ~400ns
                    # per block). Also required for the diagonal blocks,
                    # whose packed runs share psum banks: a later run's
                    # start=True clears the whole bank's has_written bits,
                    # so each run's accumulation pair must complete first.
                    for m, nlo, nhi, o in runs:
                        nc.tensor.matmul(
                            ps[:, o:o + nhi - nlo],
                            l8[g][:, :, m * MT:(m + 1) * MT],
                            r8[j][:, u, :, nlo:nhi],
                            start=True,
                            stop=False,
                            perf_mode=DR,
                        )
                        nc.tensor.matmul(
                            ps[:, o:o + nhi - nlo],
                            lb[g][:, m * MT:(m + 1) * MT],
                            rb[j][:, u, nlo:nhi],
                            start=False,
                            stop=True,
                        )
                    slot = bi % 2
                    ex = exp_t[:, slot]
                    if bi == 0:
                        # first block's exp runs as two 1024-col halves so
                        # the scalar chain starts ~0.9us earlier on the
                        # ramp (each half only needs 2 of the 4 m-tile
                        # matmul pairs; psum subtile deps let the first
                        # half fire early)
                        h = NMT * CH // 2
                        for lo, cc in ((0, col), (h, NBLK + 2)):
                            nc.scalar.activation(
                                ex[:, lo:lo + h],
                                ps[:, lo:lo + h],
                                EXP,
                                bias=bias_t[:, 0:1],
                                scale=1.0 / (LAM * LAM),
                            )
                            stt_sum(
                                ex[:, lo:lo + h // 2],
                                ex[:, lo + h // 2:lo + h],
                                cc,
                            )
                    elif diag:
                        nc.scalar.activation(
                            ex[:, :DTOT],
                            ps[:, :DTOT],
                            EXP,
                            bias=bias_t[:, 0:1],
                            scale=1.0 / (LAM * LAM),
                        )
                        stt_sum(ex[:, :DTOT // 2], ex[:, DTOT // 2:DTOT], col)
                        # diag tiles sit at exp cols {0,512} (+128) and
                        # {896,1152} (+128): two strided 2x128 views
                        d0 = ex[:, :2 * CH].rearrange(
                            "p (a b) -> p a b", a=2)[:, :, :MT]
                        d1 = ex[:, 7 * MT:11 * MT].rearrange(
                            "p (a b) -> p a b", a=2)[:, :, :MT]
                        dout = red_t[:, :2 * MT].rearrange(
                            "p (a b) -> p a b", a=2)
                        stt_sum(d0, d1, NBLK + g, out=dout)
                    else:
                        nc.scalar.activation(
                            ex,
                            ps[:],
                            EXP,
                            bias=bias_t[:, 0:1],
                            scale=1.0 / (LAM * LAM),
                        )
                        half = NMT * CH // 2
                        stt_sum(ex[:, :half], ex[:, half:], col)
            nc.scalar.dma_start(acc_d[:], acc_t[:])
    nc.compile()
    _CACHE["nc"] = nc
    return nc


def _pack_inputs(s0, s1, t0, t1):
    X0 = np.concatenate([s0, t0], axis=0).astype(np.float64)
    X1 = np.concatenate([s1, t1], axis=0).astype(np.float64)

    def gamma_of(X):
        sq = np.sum(X * X, axis=1)
        sdist = 2.0 * X.shape[0] * np.sum(sq) - 2.0 * np.sum(np.sum(X, axis=0) ** 2)
        return (X.shape[0] ** 2 - X.shape[0]) / sdist, sq

    g0, sq0 = gamma_of(X0)
    g1, sq1 = gamma_of(X1)
    c = g0 * sq0 + g1 * sq1
    cbar = c.mean()
    delta16 = -16.0 * (c - cbar)
    W0 = np.sqrt(g0) * X0  # [N, 256]
    W1 = np.sqrt(g1) * X1  # [N, 64]

    L0 = np.asarray(2.0 * LAM * W0, dtype=F8)  # [N, 256] lhs fp8
    R0 = np.asarray(LAM * W0, dtype=F8)        # [N, 256] rhs fp8
    # global rhs fp8 staged [ch, p, t, n]: feature f = t*128+p
    G8 = np.ascontiguousarray(R0.reshape(NCHUNK, CH, 2, MT).transpose(0, 3, 2, 1))
    # global rhs bf16 [ch, k, n]
    Gb = np.empty((NCHUNK, KB, CH), dtype=np.float64)
    for ch in range(NCHUNK):
        rows = slice(ch * CH, (ch + 1) * CH)
        Gb[ch, :D1] = LAM * W1[rows].T
        Gb[ch, D1] = delta16[rows]
        Gb[ch, D1 + 1] = 1.0
    Gb = Gb.astype(BF)

    def lhs_for(chunk):
        rows = slice(chunk * CH, (chunk + 1) * CH)
        a8 = np.ascontiguousarray(L0[rows].reshape(CH, 2, MT).transpose(2, 1, 0))
        ab = np.empty((KB, CH), dtype=np.float64)
        ab[:D1] = 2.0 * LAM * W1[rows].T
        ab[D1] = 1.0
        ab[D1 + 1] = delta16[rows]
        return a8, ab.astype(BF)

    bias = np.full((MT, 1), -2.0 * cbar, dtype=np.float32)

    in_maps = []
    for k in range(NCORE):
        a80, ab0 = lhs_for(k)
        a81, ab1 = lhs_for((k + 8) % NCHUNK)
        order = [(k + d) % NCHUNK for d in range(NCHUNK)]
        r8 = G8[order].reshape(8, 2, MT, 2, CH).transpose(0, 2, 1, 3, 4)
        rb = Gb[order].reshape(8, 2, KB, CH).transpose(0, 2, 1, 3)
        in_maps.append({
            "lhs8": np.ascontiguousarray(np.stack([a80, a81])),
            "lhsb": np.ascontiguousarray(np.stack([ab0, ab1])),
            "rhs8": np.ascontiguousarray(r8),
            "rhsb": np.ascontiguousarray(rb),
            "bias": bias,
        })
    return in_maps


def _combine(results):
    sgn = lambda ch: 1.0 if ch < NCHUNK // 2 else -1.0
    total = 0.0
    for k in range(NCORE):
        acc = np.asarray(results[k]["acc"], dtype=np.float64)  # [128, 17]
        colsum = acc.sum(axis=0)
        colsum[1] += colsum[NBLK + 2]  # block (g=0,d=1) was exp'd in halves
        for col in range(NBLK):
            if col < 9:
                d, row_chunk = col, k
            else:
                d, row_chunk = col - 9, (k + 8) % NCHUNK
            col_chunk = (row_chunk + d) % NCHUNK
            s = sgn(row_chunk) * sgn(col_chunk)
            if d == 0:
                # triangular diag block: full sum = 2*S - D, sign +
                total += 2.0 * colsum[col] - colsum[NBLK + (0 if col < 9 else 1)]
            else:
                total += 2.0 * s * colsum[col]
    return total / (B * B)


def kernel(s0, s1, t0, t1):
    global LAST_EXEC_NS, LAST_RESULTS
    nc = _build()
    in_maps = _pack_inputs(
        np.asarray(s0), np.asarray(s1), np.asarray(t0), np.asarray(t1)
    )
    trace = os.environ.get("JMMD_TRACE", "0") == "1"
    res = run_bass_kernel_spmd(nc, in_maps, core_ids=list(range(NCORE)), trace=trace)
    LAST_EXEC_NS = res.exec_time_ns
    LAST_RESULTS = res
    return np.float32(_combine(res.results))
